# revision 1
# baseline (speedup 1.0000x reference)
"""Trainium2 Bass kernel for the Memoroid linear-recurrence block.

Math (per batch b, fp32):
    a = sigmoid(x @ W_a + b_a)          [T, D]
    bm = x @ W_b                        [T, D]
    h_t = a_t * h_{t-1} + bm_t          (h_{-1} = h0, scan over t)
    y = gelu_tanh(h) @ W_y + x @ W_skip [T, D]
Returns (h, y).

Strategy: data-parallel over batch (8 sequences -> 8 cores). Per core,
work in transposed layout [d, t] so the recurrence maps onto the DVE
tensor_tensor_scan instruction (state = a*state + b along the free dim).
Matmuls run as float32r (fp32 bits, fast PE mode). x is transposed into
[d_in, t] tiles with PE transposes; y is computed back in natural [t, d]
orientation directly (lhsT = gelu(h)^T tiles), h is PE-transposed back.
"""

import sys

for _p in ("/opt/trn_rl_repo",):
    if _p not in sys.path:
        sys.path.insert(0, _p)

from contextlib import ExitStack

import numpy as np

import concourse.bass as bass
import concourse.bacc as bacc
import concourse.mybir as mybir
from concourse import tile
from concourse.bass_utils import run_bass_kernel_spmd
from concourse.masks import make_identity

B, T, D = 8, 4096, 1024
P = 128
KT = D // P            # 8 partition tiles along any d-dimension
TC = 256               # time-chunk length (scan tile free dim)
NCHUNK = T // TC       # 16
TSUB = TC // P         # 2  (128-row subtiles per chunk)
NO = D // 512          # 2  (512-wide output column chunks)

f32 = mybir.dt.float32
f32r = mybir.dt.float32r

_CACHE = {}


def _build(repeat=1):
    nc = bacc.Bacc()

    x_d = nc.declare_dram_parameter("x", [T, D], f32, False)
    h0_d = nc.declare_dram_parameter("h0", [D], f32, False)
    wa_d = nc.declare_dram_parameter("wa", [D, D], f32r, False)
    ba_d = nc.declare_dram_parameter("ba", [D], f32, False)
    wb_d = nc.declare_dram_parameter("wb", [D, D], f32r, False)
    wy_d = nc.declare_dram_parameter("wy", [D, D], f32r, False)
    ws_d = nc.declare_dram_parameter("ws", [D, D], f32r, False)
    h_d = nc.declare_dram_parameter("h_out", [T, D], f32, True)
    y_d = nc.declare_dram_parameter("y_out", [T, D], f32, True)

    AF = mybir.ActivationFunctionType
    ALU = mybir.AluOpType

    with tile.TileContext(nc) as tc, ExitStack() as ctx:
        wpool = ctx.enter_context(tc.tile_pool(name="weights", bufs=1))
        const_pool = ctx.enter_context(tc.tile_pool(name="const", bufs=1))
        xn_pool = ctx.enter_context(tc.tile_pool(name="xn", bufs=2))
        xt_pool = ctx.enter_context(tc.tile_pool(name="xt", bufs=2))
        sc_pool = ctx.enter_context(tc.tile_pool(name="scan", bufs=2))
        st_pool = ctx.enter_context(tc.tile_pool(name="stage", bufs=1))
        ps_pose = ctx.enter_context(tc.tile_pool(name="pose", bufs=2, space="PSUM"))
        ps_ab = ctx.enter_context(tc.tile_pool(name="ab", bufs=3, space="PSUM"))
        ps_y = ctx.enter_context(tc.tile_pool(name="ypsum", bufs=3, space="PSUM"))

        ident = const_pool.tile([P, P], f32, name="ident")
        make_identity(nc, ident[:])

        # chunk-0 x tiles are the PE's first dependency (transposes) -> DMA
        # them before any weight traffic.
        xn0 = []
        for ts in range(TSUB):
            xt_ = xn_pool.tile([P, D], f32, tag="xn", name=f"xn0_{ts}")
            nc.sync.dma_start(xt_[:], x_d[ts * P : (ts + 1) * P, :])
            xn0.append(xt_)

        # --- persistent weights: 8 partition-tiles of [128, 1024] each.
        # wa/wb are needed by chunk 0's matmuls -> load them first; wy/ws
        # are first read in phase B (one chunk later) -> defer their DMAs
        # until after chunk 0 is emitted so they don't delay the PE start.
        wa_sb, wb_sb, wy_sb, ws_sb = [], [], [], []
        for k in range(KT):
            for lst, dram, nm in ((wa_sb, wa_d, "wa"), (wb_sb, wb_d, "wb")):
                t_ = wpool.tile([P, D], f32r, tag=f"{nm}{k}", name=f"{nm}{k}")
                nc.sync.dma_start(t_[:], dram[k * P : (k + 1) * P, :])
                lst.append(t_)

        def load_late_weights():
            for k in range(KT):
                for lst, dram, nm in ((wy_sb, wy_d, "wy"), (ws_sb, ws_d, "ws")):
                    t_ = wpool.tile([P, D], f32r, tag=f"{nm}{k}", name=f"{nm}{k}")
                    nc.sync.dma_start(t_[:], dram[k * P : (k + 1) * P, :])
                    lst.append(t_)

        ba_sb, h0_sb = [], []
        for j in range(KT):
            bt = const_pool.tile([P, 1], f32, tag=f"ba{j}", name=f"ba{j}")
            nc.sync.dma_start(bt[:], ba_d[j * P : (j + 1) * P].unsqueeze(1))
            # sigmoid(z) is computed as 0.5 + 0.5*tanh(z/2) so every ACT op
            # (Tanh/Gelu_apprx_tanh/Copy) shares one activation table ->
            # no per-op table reloads. Pre-halve the bias for the tanh form.
            bh = const_pool.tile([P, 1], f32, tag=f"bah{j}", name=f"bah{j}")
            nc.scalar.mul(bh[:], bt[:], 0.5)
            ba_sb.append(bh)
            ht = const_pool.tile([P, 1], f32, tag=f"h0{j}", name=f"h0{j}")
            nc.sync.dma_start(ht[:], h0_d[j * P : (j + 1) * P].unsqueeze(1))
            h0_sb.append(ht)

        # state carried across the chunk loop (pair tiles: jp covers j=2*jp,2*jp+1)
        NP = KT // 2            # 4 j-pairs
        hT_prev = [None] * NP   # previous chunk's hT pair tiles (carry source)
        pend = None             # (xT, gT pairs, hT pairs) of prev chunk

        for rep, c in [(r, c) for r in range(repeat) for c in range(NCHUNK + 1)]:
            if c < NCHUNK:
                t0 = c * TC
                # --- load x chunk (natural layout; chunk 0 preloaded) ---
                if c == 0 and rep == 0:
                    xn = xn0
                else:
                    xn = []
                    for ts in range(TSUB):
                        xt_ = xn_pool.tile([P, D], f32, tag="xn", name=f"xn{rep}_{c}_{ts}")
                        nc.sync.dma_start(
                            xt_[:], x_d[t0 + ts * P : t0 + (ts + 1) * P, :]
                        )
                        xn.append(xt_)

                # --- transpose x -> xT [128 d_in, KT*TC] (k-major free dim) ---
                xT = xt_pool.tile([P, KT * TC], f32r, tag="xT", name=f"xT{rep}_{c}")
                xT3 = xT[:].rearrange("p (k t) -> p k t", k=KT)
                for ts in range(TSUB):
                    for k4 in range(0, KT, 4):
                        pp = ps_pose.tile([P, 4 * P], f32, tag="pose", name=f"xp{rep}_{c}_{ts}_{k4}")
                        for i in range(4):
                            k = k4 + i
                            nc.tensor.transpose(
                                pp[:, i * P : (i + 1) * P],
                                xn[ts][:, k * P : (k + 1) * P],
                                ident[:],
                            )
                        nc.scalar.copy(
                            xT3[:, k4 : k4 + 4, ts * P : (ts + 1) * P],
                            pp[:].rearrange("p (i t) -> p i t", i=4),
                        )

            if c >= 1:
                # --- phase B for chunk c-1: y matmuls + stores + h transpose ---
                xT_p, gT_p, hT_p = pend
                t0p = (c - 1) * TC
                for ts in range(TSUB):
                    for o in range(NO):
                        psY = ps_y.tile([P, 512], f32, tag="y", name=f"psY{rep}_{c-1}_{ts}_{o}")
                        for j in range(KT):
                            nc.tensor.matmul(
                                psY[:],
                                gT_p[j // 2][:, (j % 2) * TC + ts * P : (j % 2) * TC + (ts + 1) * P],
                                wy_sb[j][:, o * 512 : (o + 1) * 512],
                                start=(j == 0),
                                stop=False,
                            )
                        for k in range(KT):
                            nc.tensor.matmul(
                                psY[:],
                                xT_p[:, k * TC + ts * P : k * TC + (ts + 1) * P],
                                ws_sb[k][:, o * 512 : (o + 1) * 512],
                                start=False,
                                stop=(k == KT - 1),
                            )
                        yst = st_pool.tile(
                            [P, 512], f32, tag="yst", bufs=2, name=f"yst{rep}_{c-1}_{ts}_{o}"
                        )
                        nc.vector.tensor_copy(yst[:], psY[:])
                        nc.sync.dma_start(
                            y_d[t0p + ts * P : t0p + (ts + 1) * P, o * 512 : (o + 1) * 512],
                            yst[:],
                        )

                    # h back-transpose for this row-subtile
                    hst = st_pool.tile(
                        [P, D], f32, tag="hst", bufs=2, name=f"hst{rep}_{c-1}_{ts}"
                    )
                    for j4 in range(0, KT, 4):
                        pp = ps_pose.tile([P, 4 * P], f32, tag="pose", name=f"hp{rep}_{c-1}_{ts}_{j4}")
                        for i in range(4):
                            j = j4 + i
                            nc.tensor.transpose(
                                pp[:, i * P : (i + 1) * P],
                                hT_p[j // 2][:, (j % 2) * TC + ts * P : (j % 2) * TC + (ts + 1) * P],
                                ident[:],
                            )
                        nc.scalar.copy(
                            hst[:, j4 * P : (j4 + 4) * P], pp[:]
                        )
                    nc.sync.dma_start(
                        h_d[t0p + ts * P : t0p + (ts + 1) * P, :], hst[:]
                    )

            if c < NCHUNK:
                # --- a/b matmuls + tanh + scan + gelu, per j-pair ---
                hT_cur, gT_cur = [], []
                for jp in range(NP):
                    psA = ps_ab.tile([P, 2 * TC], f32, tag="ab", name=f"psA{rep}_{c}_{jp}")
                    psB = ps_ab.tile([P, 2 * TC], f32, tag="ab", name=f"psB{rep}_{c}_{jp}")
                    for i in range(2):
                        j = 2 * jp + i
                        for k in range(KT):
                            nc.tensor.matmul(
                                psA[:, i * TC : (i + 1) * TC],
                                wa_sb[k][:, j * P : (j + 1) * P],
                                xT[:, k * TC : (k + 1) * TC],
                                start=(k == 0),
                                stop=(k == KT - 1),
                            )
                        for k in range(KT):
                            nc.tensor.matmul(
                                psB[:, i * TC : (i + 1) * TC],
                                wb_sb[k][:, j * P : (j + 1) * P],
                                xT[:, k * TC : (k + 1) * TC],
                                start=(k == 0),
                                stop=(k == KT - 1),
                            )
                    aT = sc_pool.tile([P, 2 * TC], f32, tag=f"aT{jp}", bufs=1, name=f"aT{rep}_{c}_{jp}")
                    for i in range(2):
                        j = 2 * jp + i
                        # sigmoid(z+ba) = 0.5 + 0.5*tanh(0.5*z + 0.5*ba)
                        nc.scalar.activation(
                            aT[:, i * TC : (i + 1) * TC],
                            psA[:, i * TC : (i + 1) * TC],
                            AF.Tanh,
                            bias=ba_sb[j][:],
                            scale=0.5,
                        )
                    nc.gpsimd.tensor_scalar(
                        aT[:], aT[:], 0.5, 0.5, op0=ALU.mult, op1=ALU.add
                    )

                    hT = sc_pool.tile([P, 2 * TC], f32, tag=f"hT{jp}", name=f"hT{rep}_{c}_{jp}")
                    for i in range(2):
                        j = 2 * jp + i
                        init = (
                            h0_sb[j][:, 0:1]
                            if c == 0
                            else hT_prev[jp][:, (i + 1) * TC - 1 : (i + 1) * TC]
                        )
                        nc.vector.tensor_tensor_scan(
                            hT[:, i * TC : (i + 1) * TC],
                            aT[:, i * TC : (i + 1) * TC],
                            psB[:, i * TC : (i + 1) * TC],
                            init,
                            op0=ALU.mult,
                            op1=ALU.add,
                        )
                    gT = sc_pool.tile([P, 2 * TC], f32r, tag=f"gT{jp}", name=f"gT{rep}_{c}_{jp}")
                    nc.scalar.activation(gT[:], hT[:], AF.Gelu_apprx_tanh)
                    hT_cur.append(hT)
                    gT_cur.append(gT)

                if c == 0 and rep == 0:
                    load_late_weights()
                pend = (xT, gT_cur, hT_cur)
                hT_prev = hT_cur

    nc.finalize()
    return nc


def kernel(x, h0, W_a, b_a, W_b, W_y, W_skip):
    if "nc" not in _CACHE:
        _CACHE["nc"] = _build()
    nc = _CACHE["nc"]

    in_maps = []
    for b in range(B):
        in_maps.append(
            {
                "x": np.ascontiguousarray(np.asarray(x[b], dtype=np.float32)),
                "h0": np.ascontiguousarray(np.asarray(h0[b], dtype=np.float32)),
                "wa": np.ascontiguousarray(np.asarray(W_a, dtype=np.float32)),
                "ba": np.ascontiguousarray(np.asarray(b_a, dtype=np.float32)),
                "wb": np.ascontiguousarray(np.asarray(W_b, dtype=np.float32)),
                "wy": np.ascontiguousarray(np.asarray(W_y, dtype=np.float32)),
                "ws": np.ascontiguousarray(np.asarray(W_skip, dtype=np.float32)),
            }
        )

    res = run_bass_kernel_spmd(nc, in_maps, core_ids=list(range(B)))
    h = np.stack([r["h_out"] for r in res.results])
    y = np.stack([r["y_out"] for r in res.results])
    return h, y



# revision 2
# speedup vs baseline: 1.2833x; 1.2833x over previous
"""Trainium2 Bass kernel for the Memoroid linear-recurrence block.

Math (per batch b, fp32):
    a = sigmoid(x @ W_a + b_a)          [T, D]
    bm = x @ W_b                        [T, D]
    h_t = a_t * h_{t-1} + bm_t          (h_{-1} = h0, scan over t)
    y = gelu_tanh(h) @ W_y + x @ W_skip [T, D]
Returns (h, y).

Strategy: data-parallel over batch (8 sequences -> 8 cores). All on-chip
work happens in the transposed [d, t] orientation so the recurrence maps
onto the DVE tensor_tensor_scan instruction and NO PE transposes are
needed anywhere:
  - the host ships x already transposed (and cast to bf16) as
    [128 part, 8 k, T] so it is directly usable as the matmul moving
    operand ([d_in, t] tiles),
  - a/b are computed as [d_h, t] tiles (weights stationary),
  - y is computed transposed as well: yT[d_out, t] = W_y^T gelu(h)T +
    W_skip^T xT, consuming the scan output gT in its native layout,
  - h/y are stored transposed+bf16 to DRAM and the host transposes the
    fp32 result back.
All matmuls run in bf16 (1 cyc/row, fast weight loads); PSUM accumulates
fp32 and the scan carry stays fp32, so the end-to-end max rel-err is
~3e-3 (dominated by bf16 input rounding).
"""

import sys

for _p in ("/opt/trn_rl_repo",):
    if _p not in sys.path:
        sys.path.insert(0, _p)

from contextlib import ExitStack

import numpy as np
import ml_dtypes

import concourse.bass as bass
import concourse.bacc as bacc
import concourse.mybir as mybir
from concourse import tile
from concourse.bass_utils import run_bass_kernel_spmd

B, T, D = 8, 4096, 1024
P = 128
KT = D // P            # 8 partition tiles along any d-dimension
TB = 512               # time-block length (matmul free dim / scan length)
NB = T // TB           # 8 blocks

f32 = mybir.dt.float32
bf16 = mybir.dt.bfloat16

_CACHE = {}


def _build():
    nc = bacc.Bacc()

    # x pre-transposed on host: xt[p, k, t] = x[t, k*128+p], bf16
    xt_d = nc.declare_dram_parameter("xt", [P, KT, T], bf16, False)
    # weights pre-tiled on host: w[p, k, e] = W[k*128+p, e], bf16
    wa_d = nc.declare_dram_parameter("wa", [P, KT, D], bf16, False)
    wb_d = nc.declare_dram_parameter("wb", [P, KT, D], bf16, False)
    wy_d = nc.declare_dram_parameter("wy", [P, KT, D], bf16, False)
    ws_d = nc.declare_dram_parameter("ws", [P, KT, D], bf16, False)
    # bias/initial state tiled [p, j] fp32
    ba_d = nc.declare_dram_parameter("ba", [P, KT], f32, False)
    h0_d = nc.declare_dram_parameter("h0", [P, KT], f32, False)
    # outputs transposed: [p, j, t] bf16; host transposes back
    h_d = nc.declare_dram_parameter("h_out", [P, KT, T], bf16, True)
    y_d = nc.declare_dram_parameter("y_out", [P, KT, T], bf16, True)

    AF = mybir.ActivationFunctionType
    ALU = mybir.AluOpType

    with tile.TileContext(nc) as tc, ExitStack() as ctx:
        wpool = ctx.enter_context(tc.tile_pool(name="weights", bufs=1))
        const_pool = ctx.enter_context(tc.tile_pool(name="const", bufs=1))
        xt_pool = ctx.enter_context(tc.tile_pool(name="xt", bufs=3))
        sc_pool = ctx.enter_context(tc.tile_pool(name="scan", bufs=2))
        st_pool = ctx.enter_context(tc.tile_pool(name="stage", bufs=2))
        ps_ab = ctx.enter_context(tc.tile_pool(name="ab", bufs=2, space="PSUM"))
        ps_y = ctx.enter_context(tc.tile_pool(name="ypsum", bufs=3, space="PSUM"))

        # --- block-0 x tiles first: the PE's first dependency ---
        xt0 = xt_pool.tile([P, KT, TB], bf16, tag="xt", name="xt0")
        nc.sync.dma_start(xt0[:], xt_d[:, :, 0:TB])

        # --- a/b weights next (needed by block 0), y weights deferred ---
        wa_sb = wpool.tile([P, KT, D], bf16, tag="wa", name="wa")
        nc.sync.dma_start(wa_sb[:], wa_d[:])
        wb_sb = wpool.tile([P, KT, D], bf16, tag="wb", name="wb")
        nc.sync.dma_start(wb_sb[:], wb_d[:])

        ba_sb = const_pool.tile([P, KT], f32, name="ba")
        nc.sync.dma_start(ba_sb[:], ba_d[:])
        # sigmoid(z) = 0.5 + 0.5*tanh(z/2): pre-halve the bias, keep every
        # ACT op (Tanh/Gelu_apprx_tanh/Copy) on one activation table.
        bah = const_pool.tile([P, KT], f32, name="bah")
        nc.scalar.mul(bah[:], ba_sb[:], 0.5)
        h0_sb = const_pool.tile([P, KT], f32, name="h0")
        nc.sync.dma_start(h0_sb[:], h0_d[:])

        wy_sb = wpool.tile([P, KT, D], bf16, tag="wy", name="wy")
        ws_sb = wpool.tile([P, KT, D], bf16, tag="ws", name="ws")

        def load_late_weights():
            nc.sync.dma_start(wy_sb[:], wy_d[:])
            nc.sync.dma_start(ws_sb[:], ws_d[:])

        hT_prev = [None] * KT   # previous block's hT tiles (carry source)
        pend = None             # (xT, gT list) of previous block

        for n in range(NB + 1):
            if n < NB:
                t0 = n * TB
                if n == 0:
                    xT = xt0
                else:
                    xT = xt_pool.tile([P, KT, TB], bf16, tag="xt", name=f"xt{n}")
                    nc.sync.dma_start(xT[:], xt_d[:, :, t0 : t0 + TB])

                # --- a/b matmuls + sigmoid + scan + gelu, per j ---
                hT_cur, gT_cur = [], []
                for j in range(KT):
                    psA = ps_ab.tile([P, TB], f32, tag="a", name=f"psA{n}_{j}")
                    psB = ps_ab.tile([P, TB], f32, tag="b", name=f"psB{n}_{j}")
                    for k in range(KT):
                        nc.tensor.matmul(
                            psA[:],
                            wa_sb[:, k, j * P : (j + 1) * P],
                            xT[:, k, :],
                            start=(k == 0),
                            stop=(k == KT - 1),
                        )
                    for k in range(KT):
                        nc.tensor.matmul(
                            psB[:],
                            wb_sb[:, k, j * P : (j + 1) * P],
                            xT[:, k, :],
                            start=(k == 0),
                            stop=(k == KT - 1),
                        )
                    # sigmoid(z+ba) = 0.5 + 0.5*tanh(0.5*z + 0.5*ba)
                    aT = sc_pool.tile([P, TB], f32, tag="aT", name=f"aT{n}_{j}")
                    nc.scalar.activation(
                        aT[:], psA[:], AF.Tanh, bias=bah[:, j : j + 1], scale=0.5
                    )
                    nc.gpsimd.tensor_scalar(
                        aT[:], aT[:], 0.5, 0.5, op0=ALU.mult, op1=ALU.add
                    )
                    hT = sc_pool.tile([P, TB], f32, tag=f"hT{j}", name=f"hT{n}_{j}")
                    init = (
                        h0_sb[:, j : j + 1]
                        if n == 0
                        else hT_prev[j][:, TB - 1 : TB]
                    )
                    nc.vector.tensor_tensor_scan(
                        hT[:], aT[:], psB[:], init, op0=ALU.mult, op1=ALU.add
                    )
                    gT = sc_pool.tile([P, TB], bf16, tag=f"gT{j}", name=f"gT{n}_{j}")
                    nc.scalar.activation(gT[:], hT[:], AF.Gelu_apprx_tanh)
                    hb = st_pool.tile([P, TB], bf16, tag="hb", name=f"hb{n}_{j}")
                    nc.scalar.copy(hb[:], hT[:])
                    nc.sync.dma_start(h_d[:, j, t0 : t0 + TB], hb[:])
                    hT_cur.append(hT)
                    gT_cur.append(gT)

                if n == 0:
                    load_late_weights()

            if n >= 1:
                # --- y matmuls for block n-1 (gives the scan tail slack) ---
                xT_p, gT_p = pend
                t0p = (n - 1) * TB
                for o in range(KT):
                    psY = ps_y.tile([P, TB], f32, tag="y", name=f"psY{n-1}_{o}")
                    # skip-path first: no dependency on the scan output
                    for k in range(KT):
                        nc.tensor.matmul(
                            psY[:],
                            ws_sb[:, k, o * P : (o + 1) * P],
                            xT_p[:, k, :],
                            start=(k == 0),
                            stop=False,
                        )
                    for j in range(KT):
                        nc.tensor.matmul(
                            psY[:],
                            wy_sb[:, j, o * P : (o + 1) * P],
                            gT_p[j][:],
                            start=False,
                            stop=(j == KT - 1),
                        )
                    yb = st_pool.tile([P, TB], bf16, tag="yb", name=f"yb{n-1}_{o}")
                    nc.vector.tensor_copy(yb[:], psY[:])
                    nc.sync.dma_start(y_d[:, o, t0p : t0p + TB], yb[:])

            if n < NB:
                pend = (xT, gT_cur)
                hT_prev = hT_cur

    nc.finalize()
    return nc


def _prep_weights(W_a, b_a, W_b, W_y, W_skip):
    """Host-side: tile + cast weights once (shared across cores)."""
    def tile_w(W):
        # [D, D] -> [p, k, e] with d_in = k*128 + p
        return np.ascontiguousarray(
            np.asarray(W, dtype=np.float32)
            .reshape(KT, P, D)
            .transpose(1, 0, 2)
            .astype(ml_dtypes.bfloat16)
        )

    ba_t = np.ascontiguousarray(
        np.asarray(b_a, dtype=np.float32).reshape(KT, P).T
    )
    return {
        "wa": tile_w(W_a),
        "wb": tile_w(W_b),
        "wy": tile_w(W_y),
        "ws": tile_w(W_skip),
        "ba": ba_t,
    }


def kernel(x, h0, W_a, b_a, W_b, W_y, W_skip):
    if "nc" not in _CACHE:
        _CACHE["nc"] = _build()
    nc = _CACHE["nc"]

    shared = _prep_weights(W_a, b_a, W_b, W_y, W_skip)
    x = np.asarray(x, dtype=np.float32)
    h0 = np.asarray(h0, dtype=np.float32)

    in_maps = []
    for b in range(B):
        # x[b]: [T, D] -> xt[p, k, t] bf16
        xt = np.ascontiguousarray(
            x[b].T.reshape(KT, P, T).transpose(1, 0, 2).astype(ml_dtypes.bfloat16)
        )
        h0_t = np.ascontiguousarray(h0[b].reshape(KT, P).T)
        in_maps.append({"xt": xt, "h0": h0_t, **shared})

    res = run_bass_kernel_spmd(nc, in_maps, core_ids=list(range(B)))

    h = np.empty((B, T, D), np.float32)
    y = np.empty((B, T, D), np.float32)
    for b, r in enumerate(res.results):
        # [p, j, t] -> [t, j*128+p]
        h[b] = r["h_out"].astype(np.float32).transpose(1, 0, 2).reshape(D, T).T
        y[b] = r["y_out"].astype(np.float32).transpose(1, 0, 2).reshape(D, T).T
    return h, y


# revision 6
# speedup vs baseline: 1.2883x; 1.0038x over previous
"""Trainium2 Bass kernel for the Memoroid linear-recurrence block.

Math (per batch b, fp32):
    a = sigmoid(x @ W_a + b_a)          [T, D]
    bm = x @ W_b                        [T, D]
    h_t = a_t * h_{t-1} + bm_t          (h_{-1} = h0, scan over t)
    y = gelu_tanh(h) @ W_y + x @ W_skip [T, D]
Returns (h, y).

Strategy: data-parallel over batch (8 sequences -> 8 cores). All on-chip
work happens in the transposed [d, t] orientation so the recurrence maps
onto the DVE tensor_tensor_scan instruction and NO PE transposes are
needed anywhere:
  - the host ships x already transposed (and cast to bf16) as
    [128 part, 8 k, T] so it is directly usable as the matmul moving
    operand ([d_in, t] tiles),
  - a/b are computed as [d_h, t] tiles (weights stationary),
  - y is computed transposed as well: yT[d_out, t] = W_y^T gelu(h)T +
    W_skip^T xT, consuming the scan output gT in its native layout,
  - h/y are stored transposed+bf16 to DRAM and the host transposes the
    fp32 result back.
All matmuls run in bf16 (1 cyc/row, fast weight loads); PSUM accumulates
fp32 and the scan carry stays fp32, so the end-to-end max rel-err is
~3e-3 (dominated by bf16 input rounding).
"""

import sys

for _p in ("/opt/trn_rl_repo",):
    if _p not in sys.path:
        sys.path.insert(0, _p)

from contextlib import ExitStack

import numpy as np
import ml_dtypes

import concourse.bass as bass
import concourse.bacc as bacc
import concourse.mybir as mybir
from concourse import tile
from concourse.bass_utils import run_bass_kernel_spmd

B, T, D = 8, 4096, 1024
P = 128
KT = D // P            # 8 partition tiles along any d-dimension
TB = 512               # time-block length (matmul free dim / scan length)
NB = T // TB           # 8 blocks

f32 = mybir.dt.float32
bf16 = mybir.dt.bfloat16

_CACHE = {}


def _build():
    nc = bacc.Bacc()

    # x pre-transposed on host: xt[p, k, t] = x[t, k*128+p], bf16
    xt_d = nc.declare_dram_parameter("xt", [P, KT, T], bf16, False)
    # a/b weights pre-tiled on host OUTPUT-block-major so each j-block is
    # one small DMA and the PE can start after ~1.25 MB instead of 5 MB:
    #   w[j, p, k*128+q] = W[k*128+p, j*128+q]
    wa_d = nc.declare_dram_parameter("wa", [KT, P, D], bf16, False)
    wb_d = nc.declare_dram_parameter("wb", [KT, P, D], bf16, False)
    # y/skip weights contraction-major (loaded later, one DMA each):
    #   w[p, k, e] = W[k*128+p, e]
    wy_d = nc.declare_dram_parameter("wy", [P, KT, D], bf16, False)
    ws_d = nc.declare_dram_parameter("ws", [P, KT, D], bf16, False)
    # bias/initial state tiled [p, j] fp32
    ba_d = nc.declare_dram_parameter("ba", [P, KT], f32, False)
    h0_d = nc.declare_dram_parameter("h0", [P, KT], f32, False)
    # outputs transposed: [p, j, t] bf16; host transposes back
    h_d = nc.declare_dram_parameter("h_out", [P, KT, T], bf16, True)
    y_d = nc.declare_dram_parameter("y_out", [P, KT, T], bf16, True)

    AF = mybir.ActivationFunctionType
    ALU = mybir.AluOpType

    with tile.TileContext(nc) as tc, ExitStack() as ctx:
        wpool = ctx.enter_context(tc.tile_pool(name="weights", bufs=1))
        const_pool = ctx.enter_context(tc.tile_pool(name="const", bufs=1))
        xt_pool = ctx.enter_context(tc.tile_pool(name="xt", bufs=3))
        sc_pool = ctx.enter_context(tc.tile_pool(name="scan", bufs=2))
        st_pool = ctx.enter_context(tc.tile_pool(name="stage", bufs=2))
        ps_ab = ctx.enter_context(tc.tile_pool(name="ab", bufs=2, space="PSUM"))
        ps_y = ctx.enter_context(tc.tile_pool(name="ypsum", bufs=3, space="PSUM"))

        # --- first deps of the PE, in consumption order: wa[j=0], block-0
        # x, wb[j=0], then the remaining j blocks interleaved ---
        wa_sb, wb_sb = [], []
        for j in range(KT):
            wa_sb.append(wpool.tile([P, D], bf16, tag=f"wa{j}", name=f"wa{j}"))
            wb_sb.append(wpool.tile([P, D], bf16, tag=f"wb{j}", name=f"wb{j}"))
        nc.sync.dma_start(wa_sb[0][:], wa_d[0])
        xt0 = xt_pool.tile([P, KT, TB], bf16, tag="xt", name="xt0")
        nc.sync.dma_start(xt0[:], xt_d[:, :, 0:TB])
        nc.sync.dma_start(wb_sb[0][:], wb_d[0])
        for j in range(1, KT):
            nc.sync.dma_start(wa_sb[j][:], wa_d[j])
            nc.sync.dma_start(wb_sb[j][:], wb_d[j])

        ba_sb = const_pool.tile([P, KT], f32, name="ba")
        nc.sync.dma_start(ba_sb[:], ba_d[:])
        # sigmoid(z) = 0.5 + 0.5*tanh(z/2): pre-halve the bias, keep every
        # ACT op (Tanh/Gelu_apprx_tanh/Copy) on one activation table.
        bah = const_pool.tile([P, KT], f32, name="bah")
        nc.scalar.mul(bah[:], ba_sb[:], 0.5)
        h0_sb = const_pool.tile([P, KT], f32, name="h0")
        nc.sync.dma_start(h0_sb[:], h0_d[:])

        wy_sb = wpool.tile([P, KT, D], bf16, tag="wy", name="wy")
        ws_sb = wpool.tile([P, KT, D], bf16, tag="ws", name="ws")

        def load_late_weights():
            nc.sync.dma_start(wy_sb[:], wy_d[:])
            nc.sync.dma_start(ws_sb[:], ws_d[:])

        hT_prev = [None] * KT   # previous block's hT tiles (carry source)
        pend = None             # (xT, gT list) of previous block

        for n in range(NB + 1):
            if n < NB:
                t0 = n * TB
                if n == 0:
                    xT = xt0
                else:
                    xT = xt_pool.tile([P, KT, TB], bf16, tag="xt", name=f"xt{n}")
                    nc.sync.dma_start(xT[:], xt_d[:, :, t0 : t0 + TB])

                # --- a/b matmuls + sigmoid + scan + gelu, per j ---
                hT_cur, gT_cur = [], []
                for j in range(KT):
                    psA = ps_ab.tile([P, TB], f32, tag="a", name=f"psA{n}_{j}")
                    psB = ps_ab.tile([P, TB], f32, tag="b", name=f"psB{n}_{j}")
                    for k in range(KT):
                        nc.tensor.matmul(
                            psA[:],
                            wa_sb[j][:, k * P : (k + 1) * P],
                            xT[:, k, :],
                            start=(k == 0),
                            stop=(k == KT - 1),
                        )
                    for k in range(KT):
                        nc.tensor.matmul(
                            psB[:],
                            wb_sb[j][:, k * P : (k + 1) * P],
                            xT[:, k, :],
                            start=(k == 0),
                            stop=(k == KT - 1),
                        )
                    # sigmoid(z+ba) = 0.5 + 0.5*tanh(0.5*z + 0.5*ba)
                    aT = sc_pool.tile([P, TB], f32, tag="aT", name=f"aT{n}_{j}")
                    nc.scalar.activation(
                        aT[:], psA[:], AF.Tanh, bias=bah[:, j : j + 1], scale=0.5
                    )
                    nc.gpsimd.tensor_scalar(
                        aT[:], aT[:], 0.5, 0.5, op0=ALU.mult, op1=ALU.add
                    )
                    hT = sc_pool.tile([P, TB], f32, tag=f"hT{j}", name=f"hT{n}_{j}")
                    init = (
                        h0_sb[:, j : j + 1]
                        if n == 0
                        else hT_prev[j][:, TB - 1 : TB]
                    )
                    nc.vector.tensor_tensor_scan(
                        hT[:], aT[:], psB[:], init, op0=ALU.mult, op1=ALU.add
                    )
                    gT = sc_pool.tile([P, TB], bf16, tag=f"gT{j}", name=f"gT{n}_{j}")
                    nc.scalar.activation(gT[:], hT[:], AF.Gelu_apprx_tanh)
                    hb = st_pool.tile([P, TB], bf16, tag="hb", name=f"hb{n}_{j}")
                    nc.scalar.copy(hb[:], hT[:])
                    nc.sync.dma_start(h_d[:, j, t0 : t0 + TB], hb[:])
                    hT_cur.append(hT)
                    gT_cur.append(gT)

                if n == 0:
                    load_late_weights()

            if n >= 1:
                # --- y matmuls for block n-1 (gives the scan tail slack) ---
                xT_p, gT_p = pend
                t0p = (n - 1) * TB
                for o in range(KT):
                    psY = ps_y.tile([P, TB], f32, tag="y", name=f"psY{n-1}_{o}")
                    # skip-path first: no dependency on the scan output
                    for k in range(KT):
                        nc.tensor.matmul(
                            psY[:],
                            ws_sb[:, k, o * P : (o + 1) * P],
                            xT_p[:, k, :],
                            start=(k == 0),
                            stop=False,
                        )
                    for j in range(KT):
                        nc.tensor.matmul(
                            psY[:],
                            wy_sb[:, j, o * P : (o + 1) * P],
                            gT_p[j][:],
                            start=False,
                            stop=(j == KT - 1),
                        )
                    yb = st_pool.tile([P, TB], bf16, tag="yb", name=f"yb{n-1}_{o}")
                    nc.vector.tensor_copy(yb[:], psY[:])
                    nc.sync.dma_start(y_d[:, o, t0p : t0p + TB], yb[:])

            if n < NB:
                pend = (xT, gT_cur)
                hT_prev = hT_cur

    nc.finalize()
    return nc


def _prep_weights(W_a, b_a, W_b, W_y, W_skip):
    """Host-side: tile + cast weights once (shared across cores)."""
    def tile_w(W):
        # [D, D] -> [p, k, e] with d_in = k*128 + p (contraction-major)
        return np.ascontiguousarray(
            np.asarray(W, dtype=np.float32)
            .reshape(KT, P, D)
            .transpose(1, 0, 2)
            .astype(ml_dtypes.bfloat16)
        )

    def tile_w_j(W):
        # [D, D] -> [j, p, k*128+q] = W[k*128+p, j*128+q] (output-block-major)
        return np.ascontiguousarray(
            np.asarray(W, dtype=np.float32)
            .reshape(KT, P, KT, P)
            .transpose(2, 1, 0, 3)
            .reshape(KT, P, D)
            .astype(ml_dtypes.bfloat16)
        )

    ba_t = np.ascontiguousarray(
        np.asarray(b_a, dtype=np.float32).reshape(KT, P).T
    )
    return {
        "wa": tile_w_j(W_a),
        "wb": tile_w_j(W_b),
        "wy": tile_w(W_y),
        "ws": tile_w(W_skip),
        "ba": ba_t,
    }


def kernel(x, h0, W_a, b_a, W_b, W_y, W_skip):
    if "nc" not in _CACHE:
        _CACHE["nc"] = _build()
    nc = _CACHE["nc"]

    shared = _prep_weights(W_a, b_a, W_b, W_y, W_skip)
    x = np.asarray(x, dtype=np.float32)
    h0 = np.asarray(h0, dtype=np.float32)

    in_maps = []
    for b in range(B):
        # x[b]: [T, D] -> xt[p, k, t] bf16
        xt = np.ascontiguousarray(
            x[b].T.reshape(KT, P, T).transpose(1, 0, 2).astype(ml_dtypes.bfloat16)
        )
        h0_t = np.ascontiguousarray(h0[b].reshape(KT, P).T)
        in_maps.append({"xt": xt, "h0": h0_t, **shared})

    res = run_bass_kernel_spmd(nc, in_maps, core_ids=list(range(B)))

    h = np.empty((B, T, D), np.float32)
    y = np.empty((B, T, D), np.float32)
    for b, r in enumerate(res.results):
        # [p, j, t] -> [t, j*128+p]
        h[b] = r["h_out"].astype(np.float32).transpose(1, 0, 2).reshape(D, T).T
        y[b] = r["y_out"].astype(np.float32).transpose(1, 0, 2).reshape(D, T).T
    return h, y
